# revision 1
# baseline (speedup 1.0000x reference)
"""Trainium2 Bass kernel for nn_ContinuousGenHyperConnectionsStrang.

Contract: kernel(**inputs) takes FULL unsharded inputs (as in
reference.setup_inputs()) and returns the FULL [4096, 4, 2048] f32 output.

Strategy (8 cores, data-parallel over tokens, 512 tokens/core):
  - RMS stats + per-token generator math in token-major layout.
  - h_all = xn @ W_all.T via PE (features on partitions after PE transposes),
    with the RMS scale folded in afterwards (linear in xn).
  - 4x4 Cayley transform expS = (I-S)^-1 (I+S) computed in closed form:
    expS = [(1+p-q) I + 2S + 2S^2 + 2 Pf(S) S~ ] / (1+p+q)   (S~ = Hodge dual)
  - MLP (dominant cost, ~275 GFLOP) in f32r at full PE rate, dff processed in
    two halves of 32 k-tiles so g1 stays in SBUF (64KB/part).
  - Stream mix out = Phi@x written during the MLP; wo*y added at the end via
    gpsimd accumulate-DMA.
"""
import numpy as np

import concourse.bass as bass
import concourse.bacc as bacc
import concourse.mybir as mybir
import concourse.tile as tile
import bass_rust
from concourse.bass_utils import run_bass_kernel_spmd
from concourse.masks import make_identity
from contextlib import ExitStack

F32 = mybir.dt.float32
F32R = mybir.dt.float32r
AF = mybir.ActivationFunctionType
OP = mybir.AluOpType

NCORES = 8
B_FULL = 4096
TPC = B_FULL // NCORES          # 512 tokens per core
TT = TPC // 128                 # 4 token tiles
NS = 4                          # streams (N)
ED = 2048                       # EMBED / BLOCK
IN_DIM = NS * ED                # 8192
FD = 8192                       # DFF
KT_IN = IN_DIM // 128           # 64 k-tiles over input dim
KT_ED = ED // 128               # 16 k-tiles over embed
MT_FD = FD // 128               # 64 m-tiles over dff
KT_FD_H = 32                    # dff k-tiles per half
DT_MIN, DT_MAX = 1e-3, 1.0
DT_RANGE = DT_MAX - DT_MIN
EPS = 1.1920929e-7

PAIRS = [(0, 1), (0, 2), (0, 3), (2, 3), (1, 3), (1, 2)]
PIDX = {p: k for k, p in enumerate(PAIRS)}
U_ROWS = [4 * i + j for (i, j) in PAIRS]
L_ROWS = [4 * j + i for (i, j) in PAIRS]


def build_nc():
    nc = bacc.Bacc()
    x_d = nc.declare_dram_parameter("x", [TPC, NS, ED], F32, isOutput=False)
    wall_d = nc.declare_dram_parameter("wall", [IN_DIM, 32], F32R, isOutput=False)
    w1_d = nc.declare_dram_parameter("w1", [ED, FD], F32R, isOutput=False)
    w2_d = nc.declare_dram_parameter("w2", [FD, ED], F32R, isOutput=False)
    cvec_d = nc.declare_dram_parameter("cvec", [1, 64], F32, isOutput=False)
    out_d = nc.declare_dram_parameter("out", [TPC, NS, ED], F32, isOutput=True)

    with tile.TileContext(nc) as tc, ExitStack() as S0:
        const = S0.enter_context(tc.tile_pool(name="const", bufs=1))
        scal = S0.enter_context(tc.tile_pool(name="scal", bufs=1))
        btp = S0.enter_context(tc.tile_pool(name="btp", bufs=1))

        ident = const.tile([128, 128], F32)
        make_identity(nc, ident[:])
        ones1 = const.tile([1, 128], F32)
        nc.gpsimd.memset(ones1[:], 1.0)
        cvec_sb = const.tile([1, 64], F32)
        nc.sync.dma_start(cvec_sb[:], cvec_d[:])

        # persistent per-token scalar outputs
        C = const.tile([128, 64], F32)
        rms = scal.tile([128, TT], F32)
        hscal = scal.tile([128, 32 * TT], F32)
        ri4 = [scal.tile([128, 4], F32, tag=f"ri{t}", name=f"ri{t}") for t in range(TT)]
        wo2 = [scal.tile([128, 4], F32, tag=f"wo{t}", name=f"wo{t}") for t in range(TT)]
        PhiP = [scal.tile([128, 6], F32, tag=f"pp{t}", name=f"pp{t}") for t in range(TT)]
        PhiM = [scal.tile([128, 6], F32, tag=f"pm{t}", name=f"pm{t}") for t in range(TT)]
        PhiD = [scal.tile([128, 4], F32, tag=f"pd{t}", name=f"pd{t}") for t in range(TT)]

        # branchT [128(k), 16 kt x 512 tok] f32r, persists through MLP-1
        branchT = btp.tile([128, KT_ED * TPC], F32R)

        def phi_ap(t, i, j):
            if i == j:
                return PhiD[t][:, i:i + 1]
            if (i, j) in PIDX:
                return PhiP[t][:, PIDX[(i, j)]:PIDX[(i, j)] + 1]
            return PhiM[t][:, PIDX[(j, i)]:PIDX[(j, i)] + 1]

        with ExitStack() as SA:
            xtokp = SA.enter_context(tc.tile_pool(name="xtok", bufs=1))
            wallp = SA.enter_context(tc.tile_pool(name="wallp", bufs=1))
            xtp = SA.enter_context(tc.tile_pool(name="xtp", bufs=3))
            scrp = SA.enter_context(tc.tile_pool(name="scr", bufs=1))
            wkp = SA.enter_context(tc.tile_pool(name="wk", bufs=2))
            trps = SA.enter_context(tc.tile_pool(name="trps", bufs=3, space="PSUM"))
            haccp = SA.enter_context(tc.tile_pool(name="haccp", bufs=1, space="PSUM"))

            # broadcast cvec over partitions via PE outer product (fp32 matmul)
            cps = trps.tile([128, 512], F32, tag="cps")
            nc.tensor.matmul(cps[:, :64], ones1[:], cvec_sb[:])
            nc.vector.tensor_copy(C[:], cps[:, :64])

            wall_sb = wallp.tile([128, KT_IN * 32], F32R)
            nc.sync.dma_start(
                wall_sb[:], wall_d[:].rearrange("(kt p) m -> p kt m", p=128))

            # ---- P1: load x token-major, RMS stats ----
            x_tok = []
            ssq4 = scal.tile([128, TT * 4], F32)
            for t in range(TT):
                xt = xtokp.tile([128, NS * ED], F32, tag=f"xtok{t}", name=f"xtok{t}")
                nc.sync.dma_start(xt[:], x_d[t * 128:(t + 1) * 128])
                x_tok.append(xt)
                for j in range(NS):
                    sc = scrp.tile([128, ED], F32, tag="scr")
                    nc.scalar.activation(
                        sc[:], xt[:, j * ED:(j + 1) * ED], AF.Square,
                        accum_out=ssq4[:, t * 4 + j:t * 4 + j + 1])
            for t in range(TT):
                ssq1 = wkp.tile([128, 1], F32, tag="ssq1")
                nc.vector.reduce_sum(ssq1[:], ssq4[:, t * 4:(t + 1) * 4],
                                     axis=mybir.AxisListType.X)
                vmean = wkp.tile([128, 1], F32, tag="vmean", name=f"vmean{t}")
                nc.scalar.activation(vmean[:], ssq1[:], AF.Copy,
                                     bias=EPS, scale=1.0 / IN_DIM)
                vinv = wkp.tile([128, 1], F32, tag="vinv", name=f"vinv{t}")
                nc.vector.reciprocal(vinv[:], vmean[:])
                nc.scalar.activation(rms[:, t:t + 1], vinv[:], AF.Sqrt)

            # ---- P2: transpose x, accumulate h_all = wall.T @ xT ----
            h_ps = haccp.tile([128, 512], F32)
            for ft in range(KT_IN):
                xt_ps = trps.tile([128, 512], F32, tag="trp")
                for t in range(TT):
                    nc.tensor.transpose(
                        xt_ps[:, t * 128:(t + 1) * 128],
                        x_tok[t][:, ft * 128:(ft + 1) * 128], ident[:])
                xt_sb = xtp.tile([128, 512], F32R, tag="xt")
                nc.vector.tensor_copy(xt_sb[:], xt_ps[:])
                nc.tensor.matmul(
                    h_ps[:32, :], wall_sb[:, ft * 32:(ft + 1) * 32], xt_sb[:],
                    start=(ft == 0), stop=(ft == KT_IN - 1))

            # ---- P3: h -> token-major, apply rms ----
            hT_sb = wkp.tile([32, 512], F32, tag="hT")
            nc.scalar.activation(hT_sb[:], h_ps[:32, :], AF.Copy)
            for t in range(TT):
                hps2 = trps.tile([128, 512], F32, tag="trp")
                nc.tensor.transpose(
                    hps2[:, :32], hT_sb[:, t * 128:(t + 1) * 128], ident[:32, :32])
                nc.vector.tensor_scalar_mul(
                    hscal[:, t * 32:(t + 1) * 32], hps2[:, :32], rms[:, t:t + 1])

            # ---- P4: per-token generator scalars ----
            for t in range(TT):
                hs = hscal[:, t * 32:(t + 1) * 32]
                w = lambda cols, tg: wkp.tile([128, cols], F32, tag=tg, name=f"{tg}_{t}")

                pre_ri = w(4, "w4a")
                nc.vector.scalar_tensor_tensor(
                    pre_ri[:], hs[:, 0:4], C[:, 8:9], C[:, 0:4], OP.mult, OP.add)
                nc.scalar.activation(ri4[t][:], pre_ri[:], AF.Sigmoid)

                pre_wo = w(4, "w4b")
                nc.vector.scalar_tensor_tensor(
                    pre_wo[:], hs[:, 4:8], C[:, 9:10], C[:, 4:8], OP.mult, OP.add)
                nc.scalar.activation(wo2[t][:], pre_wo[:], AF.Sigmoid)
                nc.scalar.mul(wo2[t][:], wo2[t][:], 2.0)

                pre_dt = w(2, "w2a")
                nc.vector.tensor_add(pre_dt[:], hs[:, 8:10], C[:, 10:12])
                sg = w(2, "w2b")
                nc.scalar.activation(sg[:], pre_dt[:], AF.Sigmoid)
                dt2 = w(2, "w2c")
                nc.scalar.activation(dt2[:], sg[:], AF.Copy,
                                     bias=DT_MIN, scale=DT_RANGE)

                pre_d = w(4, "w4c")
                nc.vector.tensor_add(pre_d[:], hs[:, 10:14], C[:, 12:16])
                esp = w(4, "w4f")
                nc.scalar.activation(esp[:], pre_d[:], AF.Exp)
                dsp = w(4, "w4d")
                nc.scalar.activation(dsp[:], esp[:], AF.Ln, bias=1.0)
                dscaled = w(4, "w4e")
                nc.vector.tensor_scalar_mul(dscaled[:], dsp[:], dt2[:, 1:2])
                ehD = w(4, f"ehD{t}")
                nc.scalar.activation(ehD[:], dscaled[:], AF.Exp, scale=-0.5)

                sdiff = w(6, "w6a")
                nc.vector.tensor_sub(sdiff[:], hs[:, 14:20], hs[:, 20:26])
                spre = w(6, "w6b")
                nc.vector.tensor_add(spre[:], sdiff[:], C[:, 16:22])
                s = w(6, f"s{t}")
                nc.vector.tensor_scalar_mul(s[:], spre[:], dt2[:, 0:1])

                sq = w(6, "w6c")
                nc.vector.tensor_mul(sq[:], s[:], s[:])
                p1 = w(1, "p1")
                nc.vector.reduce_sum(p1[:], sq[:], axis=mybir.AxisListType.X)
                prod3 = w(3, "w3a")
                nc.vector.tensor_mul(prod3[:], s[:, 0:3], s[:, 3:6])
                t1 = w(1, "t1")
                nc.vector.tensor_sub(t1[:], prod3[:, 0:1], prod3[:, 1:2])
                Pf = w(1, "Pf")
                nc.vector.tensor_add(Pf[:], t1[:], prod3[:, 2:3])
                q1 = w(1, "q1")
                nc.vector.tensor_mul(q1[:], Pf[:], Pf[:])
                Dm = w(1, "Dm")
                nc.vector.tensor_add(Dm[:], p1[:], q1[:])
                D1 = w(1, "D1")
                nc.scalar.activation(D1[:], Dm[:], AF.Copy, bias=1.0)
                r0 = w(1, "r0")
                nc.vector.reciprocal(r0[:], D1[:])
                t2 = w(1, "t2")
                nc.vector.tensor_mul(t2[:], D1[:], r0[:])
                t3 = w(1, "t3")
                nc.scalar.activation(t3[:], t2[:], AF.Copy, scale=-1.0, bias=2.0)
                invD = w(1, "invD")
                nc.vector.tensor_mul(invD[:], r0[:], t3[:])

                pr1 = w(2, "pr1")
                nc.vector.tensor_mul(pr1[:], s[:, 0:2], s[:, 4:6])
                pr2 = w(4, "pr2")
                nc.vector.tensor_mul(pr2[:], s[:, 0:4], s[:, 2:6])
                pr3 = w(5, "pr3")
                nc.vector.tensor_mul(pr3[:], s[:, 0:5], s[:, 1:6])
                pr4 = w(1, "pr4")
                nc.vector.tensor_mul(pr4[:], s[:, 0:1], s[:, 5:6])

                cE = w(6, "cE")
                g01 = w(1, "g01")
                nc.vector.tensor_add(g01[:], pr1[:, 1:2], pr2[:, 2:3])
                nc.scalar.activation(cE[:, 0:1], g01[:], AF.Copy, scale=-1.0)
                nc.vector.tensor_sub(cE[:, 1:2], pr4[:, 0:1], pr3[:, 2:3])
                nc.vector.tensor_add(cE[:, 2:3], pr1[:, 0:1], pr2[:, 1:2])
                g23 = w(1, "g23")
                nc.vector.tensor_add(g23[:], pr3[:, 1:2], pr3[:, 4:5])
                nc.scalar.activation(cE[:, 3:4], g23[:], AF.Copy, scale=-1.0)
                nc.vector.tensor_sub(cE[:, 4:5], pr2[:, 3:4], pr2[:, 0:1])
                g12 = w(1, "g12")
                nc.vector.tensor_add(g12[:], pr3[:, 0:1], pr3[:, 3:4])
                nc.scalar.activation(cE[:, 5:6], g12[:], AF.Copy, scale=-1.0)

                mdiag = w(4, "mdiag")
                nc.vector.reduce_sum(mdiag[:, 0:1], sq[:, 0:3],
                                     axis=mybir.AxisListType.X)
                m1a = w(1, "m1a")
                nc.vector.reduce_sum(m1a[:], sq[:, 4:6], axis=mybir.AxisListType.X)
                nc.vector.tensor_add(mdiag[:, 1:2], m1a[:], sq[:, 0:1])
                u1 = w(1, "u1")
                nc.vector.tensor_add(u1[:], sq[:, 1:2], sq[:, 3:4])
                nc.vector.tensor_add(mdiag[:, 2:3], u1[:], sq[:, 5:6])
                nc.vector.reduce_sum(mdiag[:, 3:4], sq[:, 2:5],
                                     axis=mybir.AxisListType.X)

                st6 = w(6, "st6")
                nc.vector.tensor_mul(st6[:, 0:3], s[:, 3:6], C[:, 22:25])
                nc.vector.tensor_mul(st6[:, 3:6], s[:, 0:3], C[:, 22:25])
                o6 = w(6, "o6")
                nc.vector.scalar_tensor_tensor(
                    o6[:], st6[:], Pf[:], s[:], OP.mult, OP.add)
                nplus = w(6, "npl")
                nc.vector.tensor_add(nplus[:], cE[:], o6[:])
                nminus = w(6, "nmi")
                nc.vector.tensor_sub(nminus[:], cE[:], o6[:])

                Ppair = w(6, "Ppair")
                for k, (i, j) in enumerate(PAIRS):
                    nc.vector.tensor_mul(
                        Ppair[:, k:k + 1], ehD[:, i:i + 1], ehD[:, j:j + 1])
                nc.vector.tensor_scalar_mul(Ppair[:], Ppair[:], invD[:])
                nc.scalar.mul(Ppair[:], Ppair[:], 2.0)
                nc.vector.tensor_mul(PhiP[t][:], Ppair[:], nplus[:])
                nc.vector.tensor_mul(PhiM[t][:], Ppair[:], nminus[:])

                base = w(1, "base")
                nc.vector.tensor_sub(base[:], p1[:], q1[:])
                base1 = w(1, "base1")
                nc.scalar.activation(base1[:], base[:], AF.Copy, bias=1.0)
                m2n = w(4, "m2n")
                nc.scalar.mul(m2n[:], mdiag[:], -2.0)
                numd = w(4, "numd")
                nc.vector.tensor_scalar_add(numd[:], m2n[:], base1[:])
                e2 = w(4, "e2")
                nc.vector.tensor_mul(e2[:], ehD[:], ehD[:])
                e2i = w(4, "e2i")
                nc.vector.tensor_scalar_mul(e2i[:], e2[:], invD[:])
                nc.vector.tensor_mul(PhiD[t][:], e2i[:], numd[:])

            # ---- P5: branch = sum_n ri_n * x_n (token-major), transpose ----
            for t in range(TT):
                br = scrp.tile([128, ED], F32, tag="br")
                nc.vector.tensor_scalar_mul(
                    br[:], x_tok[t][:, 0:ED], ri4[t][:, 0:1])
                for j in range(1, NS):
                    nc.vector.scalar_tensor_tensor(
                        br[:], x_tok[t][:, j * ED:(j + 1) * ED],
                        ri4[t][:, j:j + 1], br[:], OP.mult, OP.add)
                for kg in range(KT_ED // 4):
                    bt_ps = trps.tile([128, 512], F32, tag="trp")
                    for kk in range(4):
                        kt = kg * 4 + kk
                        nc.tensor.transpose(
                            bt_ps[:, kk * 128:(kk + 1) * 128],
                            br[:, kt * 128:(kt + 1) * 128], ident[:])
                    # scatter 4 transposed tiles into branchT column blocks
                    dst = branchT[:].rearrange(
                        "p (kt tok) -> p kt tok", kt=KT_ED)[
                        :, kg * 4:(kg + 1) * 4, t * 128:(t + 1) * 128]
                    src = bt_ps[:].rearrange("p (kt tok) -> p kt tok", kt=4)
                    nc.vector.tensor_copy(dst, src)

        # ---- MLP + mix + tail ----
        with ExitStack() as SB:
            g1p = SB.enter_context(tc.tile_pool(name="g1p", bufs=1))
            yaccp = SB.enter_context(tc.tile_pool(name="yaccp", bufs=1))
            wstp = SB.enter_context(tc.tile_pool(name="wst", bufs=2))
            xrp = SB.enter_context(tc.tile_pool(name="xrp", bufs=1))
            accp = SB.enter_context(tc.tile_pool(name="accp", bufs=1))
            mlpps = SB.enter_context(
                tc.tile_pool(name="mlpps", bufs=1, space="PSUM"))

            g1T = g1p.tile([128, KT_FD_H * TPC], F32R)     # 64KB/part
            yacc = yaccp.tile([128, KT_ED * TPC], F32)     # 32KB/part
            mix_dmas = {}

            for half in range(2):
                # MLP-1: g1[mg] = gelu(W1.T @ branchT) for this dff half
                for mg in range(half * 4, half * 4 + 4):
                    g_ps = [mlpps.tile([128, 512], F32, tag=f"mm{m}", name=f"gps{mg}_{m}")
                            for m in range(8)]
                    for kt in range(KT_ED):
                        w1t = wstp.tile([128, 1024], F32R, tag="wst")
                        nc.sync.dma_start(
                            w1t[:],
                            w1_d[kt * 128:(kt + 1) * 128,
                                 mg * 1024:(mg + 1) * 1024])
                        for m in range(8):
                            nc.tensor.matmul(
                                g_ps[m][:],
                                w1t[:, m * 128:(m + 1) * 128],
                                branchT[:, kt * TPC:(kt + 1) * TPC],
                                start=(kt == 0), stop=(kt == KT_ED - 1))
                    for m in range(8):
                        mloc = (mg - half * 4) * 8 + m
                        nc.scalar.activation(
                            g1T[:, mloc * TPC:(mloc + 1) * TPC], g_ps[m][:],
                            AF.Gelu_apprx_tanh)

                # MLP-2: y += W2.T @ g1 over this half's k-tiles
                for m2g in range(2):
                    y_ps = [mlpps.tile([128, 512], F32, tag=f"mm{m}", name=f"yps{half}_{m2g}_{m}")
                            for m in range(8)]
                    for kt2 in range(KT_FD_H):
                        ktg = half * KT_FD_H + kt2
                        w2t = wstp.tile([128, 1024], F32R, tag="wst")
                        nc.sync.dma_start(
                            w2t[:],
                            w2_d[ktg * 128:(ktg + 1) * 128,
                                 m2g * 1024:(m2g + 1) * 1024])
                        for m in range(8):
                            nc.tensor.matmul(
                                y_ps[m][:],
                                w2t[:, m * 128:(m + 1) * 128],
                                g1T[:, kt2 * TPC:(kt2 + 1) * TPC],
                                start=(kt2 == 0), stop=(kt2 == KT_FD_H - 1))
                    for m in range(8):
                        m2 = m2g * 8 + m
                        dst = yacc[:, m2 * TPC:(m2 + 1) * TPC]
                        if half == 0:
                            nc.scalar.activation(dst, y_ps[m][:], AF.Copy)
                        else:
                            nc.vector.tensor_add(dst, y_ps[m][:], dst)

            # ---- P8: stream mix out = Phi @ x (overlaps MLP on DVE) ----
            for t in range(TT):
                xr = []
                for j in range(NS):
                    xj = xrp.tile([128, ED], F32, tag=f"xr{j}", name=f"xr{t}_{j}")
                    nc.sync.dma_start(xj[:], x_d[t * 128:(t + 1) * 128, j])
                    xr.append(xj)
                for i in range(NS):
                    acc = accp.tile([128, ED], F32, tag="acc")
                    nc.vector.tensor_scalar_mul(acc[:], xr[0][:], phi_ap(t, i, 0))
                    for j in range(1, NS):
                        nc.vector.scalar_tensor_tensor(
                            acc[:], xr[j][:], phi_ap(t, i, j), acc[:],
                            OP.mult, OP.add)
                    dma = nc.sync.dma_start(out_d[t * 128:(t + 1) * 128, i], acc[:])
                    mix_dmas[(t, i)] = dma

            # ---- P9: tail, out += wo2_i * y (token-major) ----
            with ExitStack() as SC:
                ytokp = SC.enter_context(tc.tile_pool(name="ytok", bufs=1))
                wyp = SC.enter_context(tc.tile_pool(name="wyp", bufs=1))
                for t in range(TT):
                    y_tok = ytokp.tile([128, ED], F32, tag="ytok")
                    for mg4 in range(4):
                        yt_ps = mlpps.tile([128, 512], F32, tag=f"mm{mg4}",
                                           name=f"ytps{t}_{mg4}")
                        for kk in range(4):
                            m2 = mg4 * 4 + kk
                            nc.tensor.transpose(
                                yt_ps[:, kk * 128:(kk + 1) * 128],
                                yacc[:, m2 * TPC + t * 128:
                                     m2 * TPC + (t + 1) * 128], ident[:])
                        nc.vector.tensor_copy(
                            y_tok[:, mg4 * 512:(mg4 + 1) * 512], yt_ps[:])
                    for i in range(NS):
                        wy = wyp.tile([128, ED], F32, tag="wy")
                        nc.scalar.activation(wy[:], y_tok[:], AF.Copy,
                                             scale=wo2[t][:, i:i + 1])
                        dma = nc.gpsimd.dma_start(
                            out_d[t * 128:(t + 1) * 128, i], wy[:],
                            accum_op=OP.add)
                        bass_rust.add_dep_helper(
                            dma.ins, mix_dmas[(t, i)].ins,
                            reason="mix partial before accum")

    nc.compile()
    return nc


_NC_CACHE = None


def _get_nc():
    global _NC_CACHE
    if _NC_CACHE is None:
        _NC_CACHE = build_nc()
    return _NC_CACHE


def prep_inputs(inputs):
    """Host-side prep: pack weights/constants for the kernel."""
    f = lambda k: np.ascontiguousarray(np.asarray(inputs[k], np.float32))
    wall = np.zeros((IN_DIM, 32), np.float32)
    wall[:, 0:4] = f("W_ri").T
    wall[:, 4:8] = f("W_wo").T
    wall[:, 8] = f("W_dtc")[0]
    wall[:, 9] = f("W_dtd")[0]
    wall[:, 10:14] = f("W_diss").T
    wall[:, 14:20] = 0.5 * f("W_conv")[U_ROWS].T
    wall[:, 20:26] = 0.5 * f("W_conv")[L_ROWS].T
    A = f("conserv_A")
    cvec = np.zeros((1, 64), np.float32)
    cvec[0, 0:4] = f("read_in_p")[:, 0]
    cvec[0, 4:8] = f("write_out_p")[:, 0]
    cvec[0, 8] = f("alpha_read_in")[0]
    cvec[0, 9] = f("alpha_write_out")[0]
    cvec[0, 10] = f("log_dt_conserv")[0] + f("b_dtc")[0]
    cvec[0, 11] = f("log_dt_diss")[0] + f("b_dtd")[0]
    cvec[0, 12:16] = f("diss_diag")
    cvec[0, 16:22] = [0.5 * (A[i, j] - A[j, i]) for (i, j) in PAIRS]
    cvec[0, 22:25] = [1.0, -1.0, 1.0]
    return {
        "wall": np.ascontiguousarray(wall),
        "w1": f("W1"),
        "w2": f("W2"),
        "cvec": cvec,
        "x": f("x"),
    }


def kernel(**inputs) -> np.ndarray:
    prep = prep_inputs(inputs)
    x = prep["x"]
    nc = _get_nc()
    in_maps = []
    for c in range(NCORES):
        in_maps.append({
            "x": np.ascontiguousarray(x[c * TPC:(c + 1) * TPC]),
            "wall": prep["wall"],
            "w1": prep["w1"],
            "w2": prep["w2"],
            "cvec": prep["cvec"],
        })
    res = run_bass_kernel_spmd(nc, in_maps, list(range(NCORES)))
    out = np.concatenate([res.results[c]["out"] for c in range(NCORES)], axis=0)
    return out.astype(np.float32)



# revision 4
# speedup vs baseline: 2.8825x; 2.8825x over previous
"""Trainium2 Bass kernel for nn_ContinuousGenHyperConnectionsStrang (v2).

Contract: kernel(**inputs) takes FULL unsharded inputs (as in
reference.setup_inputs()) and returns the FULL [4096, 4, 2048] f32 output.

Strategy (8 cores, data-parallel over tokens, 512 tokens/core):
  - x resident in SBUF as bf16; RMS stats on Act from the f32 staging DMA.
  - Per-token-tile pipeline: DMA -> stats/convert -> xT -> h (bf16 matmul)
    -> generator scalars -> branch hi/lo fp8 -> branchT.
  - MLP in fp8 e4m3 with DoubleRow perf mode (0.5 cyc/row, 2 k-tiles per
    matmul): branch split hi+lo fp8 (2-term), W1 single fp8 (x256 scale),
    g single fp8, W2 split hi+lo fp8 host-side. Measured rel err ~0.016.
  - Stream mix out = Phi@x computed on DVE/Act/Pool during the MLP phase
    (those engines idle there) and DMA'd out; tail adds wo*y via gpsimd
    accumulate-DMA per embed half, overlapping the last MLP2 eighth.
"""
import numpy as np
import ml_dtypes

import concourse.bass as bass
import concourse.bacc as bacc
import concourse.mybir as mybir
import concourse.tile as tile
import bass_rust
from concourse.bass_utils import run_bass_kernel_spmd
from concourse.masks import make_identity
from contextlib import ExitStack

F32 = mybir.dt.float32
BF16 = mybir.dt.bfloat16
F8 = mybir.dt.float8e4
AF = mybir.ActivationFunctionType
OP = mybir.AluOpType
DR = mybir.MatmulPerfMode.DoubleRow

E4 = ml_dtypes.float8_e4m3fn if hasattr(ml_dtypes, 'float8_e4m3fn') \
    else ml_dtypes.float8_e4m3
BFD = ml_dtypes.bfloat16

NCORES = 8
B_FULL = 4096
TPC = B_FULL // NCORES          # 512 tokens per core
TT = TPC // 128                 # 4 token tiles
NS = 4                          # streams
ED = 2048                       # EMBED
IN_DIM = NS * ED                # 8192
FD = 8192                       # DFF
KT_IN = IN_DIM // 128           # 64 k-tiles over input dim
KT_ED = ED // 128               # 16 k-tiles over embed
SW = 256.0                      # fp8 weight scale
ISW = 1.0 / SW
DT_MIN, DT_MAX = 1e-3, 1.0
DT_RANGE = DT_MAX - DT_MIN
EPS = 1.1920929e-7

PAIRS = [(0, 1), (0, 2), (0, 3), (2, 3), (1, 3), (1, 2)]
PIDX = {p: k for k, p in enumerate(PAIRS)}
U_ROWS = [4 * i + j for (i, j) in PAIRS]
L_ROWS = [4 * j + i for (i, j) in PAIRS]


def build_nc():
    nc = bacc.Bacc()
    x_d = nc.declare_dram_parameter("x", [TPC, NS, ED], F32, isOutput=False)
    wall_d = nc.declare_dram_parameter("wall", [IN_DIM, 32], BF16, isOutput=False)
    cvec_d = nc.declare_dram_parameter("cvec", [1, 64], F32, isOutput=False)
    # w1: (q8*8+kp) tiles of [128 k, i*512 + m]  (q8: dff eighth, kp: embed kpair)
    w1_d = nc.declare_dram_parameter("w1", [128, 128, 1024], F8, isOutput=False)
    # w2: (ebp*32+kp2) tiles of [128 k, s*1024 + i*512 + m]; kp2 over full dff
    w2_d = nc.declare_dram_parameter("w2", [128, 128, 2048], F8, isOutput=False)
    out_d = nc.declare_dram_parameter("out", [TPC, NS, ED], F32, isOutput=True)

    with tile.TileContext(nc) as tc, ExitStack() as S0:
        const = S0.enter_context(tc.tile_pool(name="const", bufs=1))
        scal = S0.enter_context(tc.tile_pool(name="scal", bufs=1))
        xbp = S0.enter_context(tc.tile_pool(name="xbp", bufs=1))
        btp = S0.enter_context(tc.tile_pool(name="btp", bufs=1))

        ident8 = const.tile([128, 128], F8)
        make_identity(nc, ident8[:])
        identb = const.tile([128, 128], BF16)
        make_identity(nc, identb[:])
        ident32 = const.tile([32, 32], F32)
        make_identity(nc, ident32[:])
        ones1 = const.tile([1, 128], F32)
        nc.gpsimd.memset(ones1[:], 1.0)
        cvec_sb = const.tile([1, 64], F32)
        nc.sync.dma_start(cvec_sb[:], cvec_d[:])
        wall_sb = const.tile([128, KT_IN * 32], BF16)
        nc.sync.dma_start(
            wall_sb[:], wall_d[:].rearrange("(kt p) m -> p kt m", p=128))
        iswc = const.tile([128, 1], F32)
        nc.gpsimd.memset(iswc[:], ISW)

        C = const.tile([128, 64], F32)
        rms = scal.tile([128, TT], F32)
        hscal = scal.tile([128, 32 * TT], F32)
        ri4 = [scal.tile([128, 4], F32, tag=f"ri{t}", name=f"ri{t}") for t in range(TT)]
        wo2 = [scal.tile([128, 4], F32, tag=f"wo{t}", name=f"wo{t}") for t in range(TT)]
        PhiP = [scal.tile([128, 6], F32, tag=f"pp{t}", name=f"pp{t}") for t in range(TT)]
        PhiM = [scal.tile([128, 6], F32, tag=f"pm{t}", name=f"pm{t}") for t in range(TT)]
        PhiD = [scal.tile([128, 4], F32, tag=f"pd{t}", name=f"pd{t}") for t in range(TT)]

        xb = xbp.tile([128, TT * IN_DIM], BF16)            # 64KB/part
        bT_hi = btp.tile([128, KT_ED * TPC], F8)           # 8KB
        bT_lo = btp.tile([128, KT_ED * TPC], F8)           # 8KB

        def phi_ap(t, i, j):
            if i == j:
                return PhiD[t][:, i:i + 1]
            if (i, j) in PIDX:
                return PhiP[t][:, PIDX[(i, j)]:PIDX[(i, j)] + 1]
            return PhiM[t][:, PIDX[(j, i)]:PIDX[(j, i)] + 1]

        # ========== Phase A: per-tile pipeline (x, stats, h, scalars, branch) ==========
        with ExitStack() as SA:
            stagep = SA.enter_context(tc.tile_pool(name="stage", bufs=3))
            xtsp = SA.enter_context(tc.tile_pool(name="xts", bufs=3))
            wkp = SA.enter_context(tc.tile_pool(name="wk", bufs=4))
            brp = SA.enter_context(tc.tile_pool(name="brp", bufs=1))
            xtps = SA.enter_context(tc.tile_pool(name="xtps", bufs=2, space="PSUM"))
            haccp = SA.enter_context(tc.tile_pool(name="hacc", bufs=2, space="PSUM"))
            ftps = SA.enter_context(tc.tile_pool(name="ftps", bufs=2, space="PSUM"))
            trp = SA.enter_context(tc.tile_pool(name="trp", bufs=1, space="PSUM"))

            # broadcast cvec over partitions via PE outer product
            cps = trp.tile([128, 512], F32, tag="trp", name="cps")
            nc.tensor.matmul(cps[:, :64], ones1[:], cvec_sb[:])
            nc.vector.tensor_copy(C[:], cps[:, :64])

            ssq4 = scal.tile([128, TT * 4], F32)

            def gen_rest():
                # generator scalars for all tiles, func-grouped to minimize
                # activation-table reloads; runs off the critical path.
                R = range(TT)
                hs = [hscal[:, t * 32:(t + 1) * 32] for t in R]
                w = lambda cols, tg: [wkp.tile([128, cols], F32, tag=tg,
                                               name=f"{tg}_{t}") for t in R]

                pre_wo = w(4, "w4b")
                for t in R:
                    nc.vector.scalar_tensor_tensor(
                        pre_wo[t][:], hs[t][:, 4:8], C[:, 9:10], C[:, 4:8],
                        OP.mult, OP.add)
                for t in R:
                    nc.scalar.activation(wo2[t][:], pre_wo[t][:], AF.Sigmoid)
                for t in R:
                    nc.scalar.mul(wo2[t][:], wo2[t][:], 2.0)

                pre_dt = w(2, "w2a")
                for t in R:
                    nc.vector.tensor_add(pre_dt[t][:], hs[t][:, 8:10], C[:, 10:12])
                sg = w(2, "w2b")
                for t in R:
                    nc.scalar.activation(sg[t][:], pre_dt[t][:], AF.Sigmoid)
                dt2 = w(2, "w2c")
                for t in R:
                    nc.scalar.activation(dt2[t][:], sg[t][:], AF.Copy,
                                         bias=DT_MIN, scale=DT_RANGE)

                pre_d = w(4, "w4c")
                for t in R:
                    nc.vector.tensor_add(pre_d[t][:], hs[t][:, 10:14], C[:, 12:16])
                esp = w(4, "w4f")
                for t in R:
                    nc.scalar.activation(esp[t][:], pre_d[t][:], AF.Exp)
                dsp = w(4, "w4d")
                for t in R:
                    nc.scalar.activation(dsp[t][:], esp[t][:], AF.Ln, bias=1.0)
                dscaled = w(4, "w4e")
                for t in R:
                    nc.vector.tensor_scalar_mul(dscaled[t][:], dsp[t][:],
                                                dt2[t][:, 1:2])
                ehD = w(4, "ehD")
                for t in R:
                    nc.scalar.activation(ehD[t][:], dscaled[t][:], AF.Exp,
                                         scale=-0.5)

                sdiff = w(6, "w6a")
                for t in R:
                    nc.vector.tensor_sub(sdiff[t][:], hs[t][:, 14:20],
                                         hs[t][:, 20:26])
                spre = w(6, "w6b")
                for t in R:
                    nc.vector.tensor_add(spre[t][:], sdiff[t][:], C[:, 16:22])
                s_ = w(6, "s6")
                for t in R:
                    nc.vector.tensor_scalar_mul(s_[t][:], spre[t][:],
                                                dt2[t][:, 0:1])

                sq6 = w(6, "w6c")
                for t in R:
                    nc.vector.tensor_mul(sq6[t][:], s_[t][:], s_[t][:])
                p1 = w(1, "p1")
                for t in R:
                    nc.vector.reduce_sum(p1[t][:], sq6[t][:],
                                         axis=mybir.AxisListType.X)
                prod3 = w(3, "w3a")
                for t in R:
                    nc.vector.tensor_mul(prod3[t][:], s_[t][:, 0:3], s_[t][:, 3:6])
                t1 = w(1, "t1")
                for t in R:
                    nc.vector.tensor_sub(t1[t][:], prod3[t][:, 0:1],
                                         prod3[t][:, 1:2])
                Pf = w(1, "Pf")
                for t in R:
                    nc.vector.tensor_add(Pf[t][:], t1[t][:], prod3[t][:, 2:3])
                q1 = w(1, "q1")
                for t in R:
                    nc.vector.tensor_mul(q1[t][:], Pf[t][:], Pf[t][:])
                Dm = w(1, "Dm")
                for t in R:
                    nc.vector.tensor_add(Dm[t][:], p1[t][:], q1[t][:])
                D1 = w(1, "D1")
                for t in R:
                    nc.scalar.activation(D1[t][:], Dm[t][:], AF.Copy, bias=1.0)
                r0 = w(1, "r0")
                for t in R:
                    nc.vector.reciprocal(r0[t][:], D1[t][:])
                t2_ = w(1, "t2")
                for t in R:
                    nc.vector.tensor_mul(t2_[t][:], D1[t][:], r0[t][:])
                t3_ = w(1, "t3")
                for t in R:
                    nc.scalar.activation(t3_[t][:], t2_[t][:], AF.Copy,
                                         scale=-1.0, bias=2.0)
                invD = w(1, "invD")
                for t in R:
                    nc.vector.tensor_mul(invD[t][:], r0[t][:], t3_[t][:])

                pr1 = w(2, "pr1")
                for t in R:
                    nc.vector.tensor_mul(pr1[t][:], s_[t][:, 0:2], s_[t][:, 4:6])
                pr2 = w(4, "pr2")
                for t in R:
                    nc.vector.tensor_mul(pr2[t][:], s_[t][:, 0:4], s_[t][:, 2:6])
                pr3 = w(5, "pr3")
                for t in R:
                    nc.vector.tensor_mul(pr3[t][:], s_[t][:, 0:5], s_[t][:, 1:6])
                pr4 = w(1, "pr4")
                for t in R:
                    nc.vector.tensor_mul(pr4[t][:], s_[t][:, 0:1], s_[t][:, 5:6])

                cE = w(6, "cE")
                g01 = w(1, "g01")
                for t in R:
                    nc.vector.tensor_add(g01[t][:], pr1[t][:, 1:2], pr2[t][:, 2:3])
                g23 = w(1, "g23")
                for t in R:
                    nc.vector.tensor_add(g23[t][:], pr3[t][:, 1:2], pr3[t][:, 4:5])
                g12 = w(1, "g12")
                for t in R:
                    nc.vector.tensor_add(g12[t][:], pr3[t][:, 0:1], pr3[t][:, 3:4])
                for t in R:
                    nc.scalar.activation(cE[t][:, 0:1], g01[t][:], AF.Copy,
                                         scale=-1.0)
                for t in R:
                    nc.scalar.activation(cE[t][:, 3:4], g23[t][:], AF.Copy,
                                         scale=-1.0)
                for t in R:
                    nc.scalar.activation(cE[t][:, 5:6], g12[t][:], AF.Copy,
                                         scale=-1.0)
                for t in R:
                    nc.vector.tensor_sub(cE[t][:, 1:2], pr4[t][:, 0:1],
                                         pr3[t][:, 2:3])
                for t in R:
                    nc.vector.tensor_add(cE[t][:, 2:3], pr1[t][:, 0:1],
                                         pr2[t][:, 1:2])
                for t in R:
                    nc.vector.tensor_sub(cE[t][:, 4:5], pr2[t][:, 3:4],
                                         pr2[t][:, 0:1])

                mdiag = w(4, "mdiag")
                m1a = w(1, "m1a")
                u1 = w(1, "u1")
                for t in R:
                    nc.vector.reduce_sum(mdiag[t][:, 0:1], sq6[t][:, 0:3],
                                         axis=mybir.AxisListType.X)
                for t in R:
                    nc.vector.reduce_sum(m1a[t][:], sq6[t][:, 4:6],
                                         axis=mybir.AxisListType.X)
                for t in R:
                    nc.vector.tensor_add(mdiag[t][:, 1:2], m1a[t][:],
                                         sq6[t][:, 0:1])
                for t in R:
                    nc.vector.tensor_add(u1[t][:], sq6[t][:, 1:2], sq6[t][:, 3:4])
                for t in R:
                    nc.vector.tensor_add(mdiag[t][:, 2:3], u1[t][:],
                                         sq6[t][:, 5:6])
                for t in R:
                    nc.vector.reduce_sum(mdiag[t][:, 3:4], sq6[t][:, 2:5],
                                         axis=mybir.AxisListType.X)

                st6 = w(6, "st6")
                for t in R:
                    nc.vector.tensor_mul(st6[t][:, 0:3], s_[t][:, 3:6],
                                         C[:, 22:25])
                for t in R:
                    nc.vector.tensor_mul(st6[t][:, 3:6], s_[t][:, 0:3],
                                         C[:, 22:25])
                o6 = w(6, "o6")
                for t in R:
                    nc.vector.scalar_tensor_tensor(
                        o6[t][:], st6[t][:], Pf[t][:], s_[t][:], OP.mult, OP.add)
                nplus = w(6, "npl")
                for t in R:
                    nc.vector.tensor_add(nplus[t][:], cE[t][:], o6[t][:])
                nminus = w(6, "nmi")
                for t in R:
                    nc.vector.tensor_sub(nminus[t][:], cE[t][:], o6[t][:])

                Ppair = w(6, "Ppair")
                for t in R:
                    for k, (i, j) in enumerate(PAIRS):
                        nc.vector.tensor_mul(
                            Ppair[t][:, k:k + 1], ehD[t][:, i:i + 1],
                            ehD[t][:, j:j + 1])
                for t in R:
                    nc.vector.tensor_scalar_mul(Ppair[t][:], Ppair[t][:],
                                                invD[t][:])
                for t in R:
                    nc.scalar.mul(Ppair[t][:], Ppair[t][:], 2.0)
                for t in R:
                    nc.vector.tensor_mul(PhiP[t][:], Ppair[t][:], nplus[t][:])
                for t in R:
                    nc.vector.tensor_mul(PhiM[t][:], Ppair[t][:], nminus[t][:])

                base = w(1, "base")
                for t in R:
                    nc.vector.tensor_sub(base[t][:], p1[t][:], q1[t][:])
                base1 = w(1, "base1")
                for t in R:
                    nc.scalar.activation(base1[t][:], base[t][:], AF.Copy,
                                         bias=1.0)
                m2n = w(4, "m2n")
                for t in R:
                    nc.scalar.mul(m2n[t][:], mdiag[t][:], -2.0)
                numd = w(4, "numd")
                for t in R:
                    nc.vector.tensor_scalar_add(numd[t][:], m2n[t][:],
                                                base1[t][:])
                e2 = w(4, "e2")
                for t in R:
                    nc.vector.tensor_mul(e2[t][:], ehD[t][:], ehD[t][:])
                e2i = w(4, "e2i")
                for t in R:
                    nc.vector.tensor_scalar_mul(e2i[t][:], e2[t][:], invD[t][:])
                for t in R:
                    nc.vector.tensor_mul(PhiD[t][:], e2i[t][:], numd[t][:])

            for t in range(TT):
                xoff = t * IN_DIM
                h_ps = haccp.tile([128, 512], F32, tag="hps", name=f"hps{t}")
                for jh in range(2):
                    for j2 in range(2):
                        j = 2 * jh + j2
                        stage = stagep.tile([128, ED], F32, tag="stage",
                                            name=f"stage{t}_{j}")
                        nc.sync.dma_start(
                            stage[:], x_d[t * 128:(t + 1) * 128, j])
                        # bf16 conversion on Pool (per stream chunk)
                        nc.gpsimd.tensor_copy(
                            xb[:, xoff + j * ED:xoff + (j + 1) * ED], stage[:])
                    # transposes + h accumulation (8 kt per PSUM bank)
                    for kbl in range(4):
                        kb = jh * 4 + kbl
                        bank = xtps.tile([128, 1024], BF16, tag="xtb")
                        for k8 in range(8):
                            kt = kb * 8 + k8
                            nc.tensor.transpose(
                                bank[:, k8 * 128:(k8 + 1) * 128],
                                xb[:, xoff + kt * 128:xoff + (kt + 1) * 128],
                                identb[:])
                        xts = xtsp.tile([128, 1024], BF16, tag="xts")
                        nc.vector.tensor_copy(xts[:], bank[:])
                        for k8 in range(8):
                            kt = kb * 8 + k8
                            nc.tensor.matmul(
                                h_ps[:32, 0:128],
                                wall_sb[:, kt * 32:(kt + 1) * 32],
                                xts[:, k8 * 128:(k8 + 1) * 128],
                                start=(kt == 0),
                                stop=(kt == KT_IN - 1))

                # RMS stats from bf16 x (Act square + accum register),
                # after the xts drains so they don't gate the h matmuls
                for j in range(NS):
                    sq = wkp.tile([128, ED], F32, tag="sqscr",
                                  name=f"sq{t}_{j}")
                    nc.scalar.activation(
                        sq[:], xb[:, xoff + j * ED:xoff + (j + 1) * ED],
                        AF.Square,
                        accum_out=ssq4[:, t * 4 + j:t * 4 + j + 1])
                # rms finalize
                ssq1 = wkp.tile([128, 1], F32, tag="ssq1", name=f"ssq1_{t}")
                nc.vector.reduce_sum(ssq1[:], ssq4[:, t * 4:(t + 1) * 4],
                                     axis=mybir.AxisListType.X)
                vmean = wkp.tile([128, 1], F32, tag="vmean", name=f"vmean{t}")
                nc.scalar.activation(vmean[:], ssq1[:], AF.Copy,
                                     bias=EPS, scale=1.0 / IN_DIM)
                vinv = wkp.tile([128, 1], F32, tag="vinv", name=f"vinv{t}")
                nc.vector.reciprocal(vinv[:], vmean[:])
                nc.scalar.activation(rms[:, t:t + 1], vinv[:], AF.Sqrt)

                # h -> token-major, apply rms
                hT_sb = wkp.tile([32, 128], F32, tag="hT", name=f"hT{t}")
                nc.scalar.activation(hT_sb[:], h_ps[:32, 0:128], AF.Copy)
                hps2 = trp.tile([128, 512], F32, tag="trp", name=f"hps2_{t}")
                nc.tensor.transpose(hps2[:, :32], hT_sb[:], ident32[:])
                nc.vector.tensor_scalar_mul(
                    hscal[:, t * 32:(t + 1) * 32], hps2[:, :32], rms[:, t:t + 1])

                # read-in gates for this tile (rest of generator scalars
                # is batched in gen_rest() off the critical path)
                pre_ri = wkp.tile([128, 4], F32, tag="w4a", name=f"w4a_{t}")
                nc.vector.scalar_tensor_tensor(
                    pre_ri[:], hscal[:, t * 32:t * 32 + 4], C[:, 8:9],
                    C[:, 0:4], OP.mult, OP.add)
                nc.scalar.activation(ri4[t][:], pre_ri[:], AF.Sigmoid)

                # branch = sum_n ri_n * x_n (DVE chain, bf16 acc);
                # split hi/lo fp8 (hi on Act, lo on DVE)
                br = brp.tile([128, ED], BF16, tag="br", name=f"br{t}")
                nc.vector.tensor_scalar_mul(
                    br[:], xb[:, xoff:xoff + ED], ri4[t][:, 0:1])
                for j in range(1, NS):
                    nc.vector.scalar_tensor_tensor(
                        br[:], xb[:, xoff + j * ED:xoff + (j + 1) * ED],
                        ri4[t][:, j:j + 1], br[:], OP.mult, OP.add)
                bhi = brp.tile([128, ED], F8, tag="bhi", name=f"bhi{t}")
                nc.scalar.activation(bhi[:], br[:], AF.Copy)
                blo = brp.tile([128, ED], F8, tag="blo", name=f"blo{t}")
                nc.vector.tensor_sub(blo[:], br[:], bhi[:])
                # transpose 4 kt x (hi,lo) per PSUM bank (stride-2 fp8)
                for kb in range(KT_ED // 4):
                    bank = ftps.tile([128, 2048], F8, tag="ftb")
                    bview = bank[:].rearrange(
                        "p (g n two) -> p g n two", g=8, two=2)
                    for k4 in range(4):
                        kt = kb * 4 + k4
                        nc.tensor.transpose(
                            bview[:, k4 * 2, :, 0],
                            bhi[:, kt * 128:(kt + 1) * 128], ident8[:])
                        nc.tensor.transpose(
                            bview[:, k4 * 2 + 1, :, 0],
                            blo[:, kt * 128:(kt + 1) * 128], ident8[:])
                    # drains: hi 4kt (DVE), lo 4kt (Act)
                    for s8, dst in ((0, bT_hi), (1, bT_lo)):
                        src = bank[:].rearrange(
                            "p (k4 s8 n two) -> p k4 s8 n two",
                            k4=4, s8=2, two=2)[:, :, s8, :, 0]
                        dstv = dst[:].rearrange(
                            "p (kt tok) -> p kt tok", kt=KT_ED)[
                            :, kb * 4:(kb + 1) * 4, t * 128:(t + 1) * 128]
                        if s8 == 0:
                            nc.vector.tensor_copy(dstv, src)
                        else:
                            nc.scalar.activation(dstv, src, AF.Copy)

            gen_rest()

        # ================= Phase B: MLP + mix + tail =================
        # Units of 4 PSUM-bank groups, alternating between two bank sets so
        # the PE never waits on drains.
        mix_scr = {}
        unit_ctr = [0]
        with ExitStack() as SM:
            g1p = SM.enter_context(tc.tile_pool(name="g1p", bufs=1))
            yaccp = SM.enter_context(tc.tile_pool(name="yaccp", bufs=1))
            g1 = g1p.tile([128, 64 * TPC], F8)             # 32KB (full dff)
            yT = yaccp.tile([128, TT * ED], BF16)          # 16KB token-major y
            w1sp = SM.enter_context(tc.tile_pool(name="w1s", bufs=8))
            w2sp = SM.enter_context(tc.tile_pool(name="w2s", bufs=6))
            oscrp = SM.enter_context(tc.tile_pool(name="oscr", bufs=34))
            fstp = SM.enter_context(tc.tile_pool(name="fst", bufs=4))
            mlpps = SM.enter_context(tc.tile_pool(name="mlpps", bufs=1, space="PSUM"))

            def mlp1_eighth(q8):
                # one dff eighth: 4 m-tiles (512 dff), groups (c, mpair)
                base = (unit_ctr[0] % 2) * 4
                unit_ctr[0] += 1
                qh = q8              # dff eighth (g1 holds full dff)
                banks = [mlpps.tile([128, 512], F32, tag=f"bk{base + g}",
                                    name=f"m1b_{q8}_{g}") for g in range(4)]
                for kp in range(8):
                    w1t = w1sp.tile([128, 1024], F8, tag="w1t",
                                    name=f"w1t_{q8}_{kp}")
                    nc.sync.dma_start(w1t[:], w1_d[q8 * 8 + kp])
                    lhs_all = w1t[:].rearrange("p (i m) -> p i m", i=2)
                    for c in range(2):
                        rhs_hi = bT_hi[:].rearrange(
                            "p (kt n) -> p kt n", kt=KT_ED)[
                            :, 2 * kp:2 * kp + 2, c * 256:(c + 1) * 256]
                        rhs_lo = bT_lo[:].rearrange(
                            "p (kt n) -> p kt n", kt=KT_ED)[
                            :, 2 * kp:2 * kp + 2, c * 256:(c + 1) * 256]
                        for mp in range(2):
                            bank = banks[c * 2 + mp]
                            for si, r in ((0, rhs_hi), (1, rhs_lo)):
                                for mi in range(2):
                                    mloc = mp * 2 + mi
                                    nc.tensor.matmul(
                                        bank[:, mi * 256:(mi + 1) * 256],
                                        lhs_all[:, :, mloc * 128:(mloc + 1) * 128],
                                        r, perf_mode=DR,
                                        start=(kp == 0 and si == 0 and mi == 0),
                                        stop=(kp == 7 and si == 1 and mi == 1))
                # gelu drains -> g1 fp8
                for c in range(2):
                    for mp in range(2):
                        dst = g1[:].rearrange(
                            "p (m tok) -> p m tok", m=64)[
                            :, qh * 4 + mp * 2:qh * 4 + mp * 2 + 2,
                            c * 256:(c + 1) * 256]
                        src = banks[c * 2 + mp][:].rearrange(
                            "p (m n) -> p m n", m=2)
                        nc.scalar.activation(dst, src, AF.Gelu_apprx_tanh,
                                             scale=ISW)

            def mlp2_unit(ebp):
                # token-major MLP2 for one 512-wide embed block:
                # out[tok, e] accumulated over the full dff; stationary = g1
                # slice [dffk, 2, tok], moving = W2 tile [dffk, 2, 512].
                base = (unit_ctr[0] % 2) * 4
                unit_ctr[0] += 1
                banks = [mlpps.tile([128, 512], F32, tag=f"bk{base + g}",
                                    name=f"m2b_{ebp}_{g}") for g in range(4)]
                lhs_g = g1[:].rearrange("p (kt n) -> p kt n", kt=64)
                for kp2 in range(32):
                    w2t = w2sp.tile([128, 2048], F8, tag="w2t",
                                    name=f"w2t_{ebp}_{kp2}")
                    nc.sync.dma_start(w2t[:], w2_d[ebp * 32 + kp2])
                    w2v = w2t[:].rearrange("p (s i m) -> p s i m", s=2, i=2)
                    for t in range(TT):
                        lhsT = lhs_g[:, 2 * kp2:2 * kp2 + 2,
                                     t * 128:(t + 1) * 128]
                        for si in range(2):
                            for e2 in range(2):
                                nc.tensor.matmul(
                                    banks[t][:, e2 * 256:(e2 + 1) * 256],
                                    lhsT,
                                    w2v[:, si, :, e2 * 256:(e2 + 1) * 256],
                                    perf_mode=DR,
                                    start=(kp2 == 0 and si == 0 and e2 == 0),
                                    stop=(kp2 == 31 and si == 1 and e2 == 1))
                # drain to token-major yT (bf16, scaled 1/SW)
                for t in range(TT):
                    nc.scalar.activation(
                        yT[:, t * ED + ebp * 512:t * ED + (ebp + 1) * 512],
                        banks[t][:], AF.Copy, scale=ISW)

            def mix_slices(ebp, form_a):
                # acc_{t,i} = sum_j Phi_ij x_j for this embed block (bf16).
                # form A (MLP1 window): Act init + 3 DVE STT passes.
                # form B (MLP2 window): Act init/term3 + 2 DVE STT + DVE add.
                for t in range(TT):
                    for i in range(NS):
                        scr = oscrp.tile([128, 512], BF16, tag="oscr",
                                         name=f"oscr{ebp}_{t}_{i}")
                        mix_scr[(ebp, t, i)] = scr
                        xsl = lambda j: xb[:, t * IN_DIM + j * ED + ebp * 512:
                                           t * IN_DIM + j * ED +
                                           (ebp + 1) * 512]
                        nc.scalar.activation(scr[:], xsl(0), AF.Copy,
                                             scale=phi_ap(t, i, 0))
                        nc.vector.scalar_tensor_tensor(
                            scr[:], xsl(1), phi_ap(t, i, 1), scr[:],
                            OP.mult, OP.add)
                        nc.vector.scalar_tensor_tensor(
                            scr[:], xsl(2), phi_ap(t, i, 2), scr[:],
                            OP.mult, OP.add)
                        if form_a:
                            nc.vector.scalar_tensor_tensor(
                                scr[:], xsl(3), phi_ap(t, i, 3), scr[:],
                                OP.mult, OP.add)
                        else:
                            tm3 = oscrp.tile([128, 512], BF16, tag="tm3",
                                             name=f"tm3_{ebp}_{t}_{i}", bufs=3)
                            nc.scalar.activation(tm3[:], xsl(3), AF.Copy,
                                                 scale=phi_ap(t, i, 3))
                            nc.vector.tensor_add(scr[:], scr[:], tm3[:])

            def tail_ebp(ebp):
                # out = mix + wo_i * y for this embed block; single DMA write
                for t in range(TT):
                    for i in range(NS):
                        fst = fstp.tile([128, 512], F32, tag="fst",
                                        name=f"fst_{ebp}_{t}_{i}")
                        nc.vector.scalar_tensor_tensor(
                            fst[:],
                            yT[:, t * ED + ebp * 512:t * ED + (ebp + 1) * 512],
                            wo2[t][:, i:i + 1], mix_scr[(ebp, t, i)][:],
                            OP.mult, OP.add)
                        nc.gpsimd.dma_start(
                            out_d[t * 128:(t + 1) * 128, i,
                                  ebp * 512:(ebp + 1) * 512], fst[:])

            for q8 in range(10):
                mlp1_eighth(q8)
            mix_slices(0, form_a=True)
            for q8 in range(10, 13):
                mlp1_eighth(q8)
            mix_slices(1, form_a=True)
            for q8 in range(13, 16):
                mlp1_eighth(q8)
            mlp2_unit(0)
            tail_ebp(0)
            mix_slices(2, form_a=False)
            mlp2_unit(1)
            tail_ebp(1)
            mix_slices(3, form_a=False)
            mlp2_unit(2)
            tail_ebp(2)
            mlp2_unit(3)
            tail_ebp(3)

    nc.compile()
    return nc


_NC_CACHE = None


def _get_nc():
    global _NC_CACHE
    if _NC_CACHE is None:
        _NC_CACHE = build_nc()
    return _NC_CACHE


def prep_inputs(inputs):
    """Host-side prep: pack weights/constants for the kernel."""
    f = lambda k: np.ascontiguousarray(np.asarray(inputs[k], np.float32))
    wall = np.zeros((IN_DIM, 32), np.float32)
    wall[:, 0:4] = f("W_ri").T
    wall[:, 4:8] = f("W_wo").T
    wall[:, 8] = f("W_dtc")[0]
    wall[:, 9] = f("W_dtd")[0]
    wall[:, 10:14] = f("W_diss").T
    wall[:, 14:20] = 0.5 * f("W_conv")[U_ROWS].T
    wall[:, 20:26] = 0.5 * f("W_conv")[L_ROWS].T
    A = f("conserv_A")
    cvec = np.zeros((1, 64), np.float32)
    cvec[0, 0:4] = f("read_in_p")[:, 0]
    cvec[0, 4:8] = f("write_out_p")[:, 0]
    cvec[0, 8] = f("alpha_read_in")[0]
    cvec[0, 9] = f("alpha_write_out")[0]
    cvec[0, 10] = f("log_dt_conserv")[0] + f("b_dtc")[0]
    cvec[0, 11] = f("log_dt_diss")[0] + f("b_dtd")[0]
    cvec[0, 12:16] = f("diss_diag")
    cvec[0, 16:22] = [0.5 * (A[i, j] - A[j, i]) for (i, j) in PAIRS]
    cvec[0, 22:25] = [1.0, -1.0, 1.0]

    # W1 fp8 pack: [q8*8+kp, k, i*512+m] = e4m3(SW*W1[(2kp+i)*128+k, q8*512+m])
    W1s = f("W1") * SW                                   # [2048, 8192]
    w1r = W1s.reshape(8, 2, 128, 16, 512)                # kp, i, k, q8, m
    w1p = np.ascontiguousarray(
        w1r.transpose(3, 0, 2, 1, 4).reshape(128, 128, 1024)).astype(E4)

    # W2 fp8 hi/lo pack:
    # [ebp*32+kp2, k, s*1024+i*512+m] = Ws[(2*kp2+i)*128+k, ebp*512+m]
    W2s = f("W2") * SW                                   # [8192, 2048]
    W2hi = W2s.astype(E4).astype(np.float32)
    W2lo = W2s - W2hi
    packed = np.stack(
        [W2hi.reshape(32, 2, 128, 4, 512),
         W2lo.reshape(32, 2, 128, 4, 512)],
        axis=0)                                          # s, kp2, i, k, ebp, m
    w2p = np.ascontiguousarray(
        packed.transpose(4, 1, 3, 0, 2, 5).reshape(128, 128, 2048)).astype(E4)

    return {
        "wall": wall.astype(BFD),
        "w1": w1p,
        "w2": w2p,
        "cvec": cvec,
        "x": f("x"),
    }


def kernel(**inputs) -> np.ndarray:
    prep = prep_inputs(inputs)
    x = prep["x"]
    nc = _get_nc()
    in_maps = []
    for c in range(NCORES):
        in_maps.append({
            "x": np.ascontiguousarray(x[c * TPC:(c + 1) * TPC]),
            "wall": prep["wall"],
            "w1": prep["w1"],
            "w2": prep["w2"],
            "cvec": prep["cvec"],
        })
    res = run_bass_kernel_spmd(nc, in_maps, list(range(NCORES)))
    out = np.concatenate([res.results[c]["out"] for c in range(NCORES)], axis=0)
    return out.astype(np.float32)
